# revision 3
# baseline (speedup 1.0000x reference)
"""GCNSimple v2: 8-core data-parallel, 2 launches.

A: embed h=(atom@WE+b)*io -> bf16 table window; ew=exp(-||r||^2) for all
   edge slots (tier1 + overflow layout); iid=rsqrt(indeg) out.
B: tier1 fixed-4 (node,window) gather cells -> ew-mult -> 2-level tree ->
   SBUF accumulate; overflow edges gather + scatter-add into DRAM ACC;
   epilogue x=relu((agg@W1)*ii+b1), z=(x@W2)*io; dense pooling matmul
   PART[512] = sum_s z_s * K'[s,:] (K' built on host from device ew/iid).
Host: index/layout preprocessing, K' bincount, final 8-way partial sum.
"""
import sys
sys.path.insert(0, "/opt/trn_rl_repo")
import numpy as np
import ml_dtypes

import concourse.bacc as bacc
import concourse.mybir as mybir
from concourse import ap_utils
from concourse.bass import MemorySpace, AP as _APc, IndirectOffsetOnAxis
from concourse.tile import TileContext, add_dep_helper
from concourse._compat import exact_div
from concourse.bass_utils import run_bass_kernel_spmd

P = 128
NCORES = 8
N_NODES = 200_000
N_EDGES = 6_400_000
N_GRAPHS = 512
GPC = N_GRAPHS // NCORES
F_IN = 92
W = 10
K1 = 4                  # tier1 slots per (node, window)
CPT = 64                # slot-cols per tier1 gather chunk
OVC = 8192              # target overflow idx per chunk
NQ = 4
FP = np.float32
BF = ml_dtypes.bfloat16
ROWE = 128              # table row elems (bf16) = 256B


# ---------------------------------------------------------------- raw gather
def dma_gather_raw(gp, out_ap, in_ap, idxs_ap, num_idxs, elem_size, elem_step,
                   queue_num=0):
    gp._assert_queue_num(queue_num)
    assert idxs_ap.dtype == mybir.dt.int16
    assert in_ap.dtype == out_ap.dtype
    assert in_ap.space == MemorySpace.DRAM
    assert ap_utils.ap_is_contiguous(in_ap.ap[1:])
    assert ap_utils.ap_is_contiguous(out_ap.ap[1:])
    assert ap_utils.ap_is_contiguous(idxs_ap.ap[1:])
    assert in_ap.ap[-1][1] == out_ap.ap[-1][1] == elem_size
    assert out_ap.ap[0][1] * out_ap.ap[1][1] == (num_idxs + 127) // 128 * 128
    assert in_ap.ap[0][0] == elem_step
    stride_bytes_256 = exact_div(elem_step * mybir.dt.size(in_ap.dtype), 256)
    return gp.add_instruction(
        mybir.InstDMAGatherAnt(
            name=gp.bass.get_next_instruction_name(),
            ins=[*gp.lower_ap_dma(in_ap, for_custom_bir_dma=True),
                 gp.lower_ap(idxs_ap),
                 gp.lower_val_access(gp.to_reg(num_idxs))],
            outs=[gp.lower_ap(out_ap)],
            transpose=False, num_idxs=num_idxs, elem_size=elem_size,
            stride_bytes_256=stride_bytes_256, gen_mode=0,
            single_packet=False, queue_num=queue_num,
            sbuf_tokens_per_rank=0, sbuf_free_dim_per_rank=0,
            sbuf_free_dim_pad_per_rank=0, sbuf_byte_offset=0,
        ))


def wrap_idx16(logical_idx, num_idxs):
    w = np.zeros((16, num_idxs // 16), np.int16)
    ar = np.arange(num_idxs)
    w[ar % 16, ar // 16] = logical_idx.astype(np.int16)
    return np.tile(w, (8, 1))


def _bc(t_ap, dims):
    return _APc(t_ap.tensor, t_ap.offset, [list(d) for d in dims])


# ---------------------------------------------------------------- host plan
def build_plan2(src, dst, graph_ids, r_in):
    src = np.asarray(src, np.int64)
    dst = np.asarray(dst, np.int64)
    gid = np.asarray(graph_ids, np.int64)
    r_in = np.asarray(r_in, FP)

    corenode = gid // GPC
    first = np.searchsorted(corenode, np.arange(NCORES + 1))
    cnt_core = np.diff(first)
    NPADU = int((cnt_core.max() + P - 1) // P * P)
    COLS = NPADU // P
    NTAB = NPADU + 1
    assert NTAB < 32767
    slot = np.arange(N_NODES) - first[corenode]

    outdeg = np.bincount(src, minlength=N_NODES).astype(np.int32)
    indeg = np.bincount(dst, minlength=N_NODES).astype(np.int32)

    ecore = corenode[dst]
    ewin = corenode[src]
    WCOLS = COLS * K1               # tier1 cols per window region
    T1COLS = NCORES * WCOLS         # tier1 cols per core

    # ---- per-core tier1 fill + overflow edge lists (sorted by w, dst)
    pre = []
    ovcnt = np.zeros((NCORES, NCORES), np.int64)   # [core, window] -> edges
    for c in range(NCORES):
        em = np.nonzero(ecore == c)[0]
        ew_, ed_ = ewin[em], dst[em]
        order = np.lexsort((ed_, ew_))
        em, ew_, ed_ = em[order], ew_[order], ed_[order]
        key = ew_ * N_NODES + ed_
        chg = np.empty(len(em), bool)
        chg[0] = True
        chg[1:] = key[1:] != key[:-1]
        gstart = np.where(chg, np.arange(len(em)), 0)
        np.maximum.accumulate(gstart, out=gstart)
        j = np.arange(len(em)) - gstart
        t1m = j < K1
        ovm = ~t1m
        ovcnt[c] = np.bincount(ew_[ovm], minlength=NCORES)
        pre.append(dict(em=em, ew=ew_, ed=ed_, j=j, t1m=t1m, ovm=ovm))

    # ---- uniform overflow schedule (PAIR-aligned): same-dst edges sit in
    # aligned col pairs; scatter happens per pair sum. Chunk unit = pairs.
    # per (core, w): pairs = sum over cells of ceil(ovn/2)
    ovpairs = np.zeros((NCORES, NCORES), np.int64)
    for c in range(NCORES):
        pc = pre[c]
        ew_, ed_, ovm = pc["ew"], pc["ed"], pc["ovm"]
        emo_w, edo_w = ew_[ovm], ed_[ovm]
        key = emo_w * N_NODES + edo_w
        if len(key):
            cnt = np.bincount(
                np.unique(key, return_inverse=True)[1])
            uw = emo_w[np.unique(key, return_index=True)[1]]
            prs = (cnt + 1) // 2
            ovpairs[c] = np.bincount(uw, weights=prs,
                                     minlength=NCORES).astype(np.int64)
    OVCP = OVC // 2                   # pairs per chunk target
    ovsched = []                      # list of (window, npad_cols) cols EVEN
    for w_ in range(NCORES):
        n = int(ovpairs[:, w_].max())
        done = 0
        while done < n:
            take = min(OVCP, n - done)
            ovsched.append((w_, 2 * ((take + P - 1) // P)))
            done += take
    OVTOT = sum(nc_ for _, nc_ in ovsched)
    RCOLS = T1COLS + OVTOT
    ovbase = np.zeros(len(ovsched), np.int64)     # col base within ov region
    b = 0
    for i, (w_, nc_) in enumerate(ovsched):
        ovbase[i] = b
        b += nc_
    # per-window chunk id list and base edge offsets (uniform)
    wchunks = {w_: [i for i, (ww, _) in enumerate(ovsched) if ww == w_]
               for w_ in range(NCORES)}

    # ---- per-core streams
    cores = []
    epos_all = np.full(N_EDGES, -1, np.int64)   # position in core's stream
    for c in range(NCORES):
        pc = pre[c]
        em, ew_, ed_, t1m, ovm, j = (pc["em"], pc["ew"], pc["ed"], pc["t1m"],
                                     pc["ovm"], pc["j"])
        # tier1
        sd = slot[ed_[t1m]]
        pos1 = (ew_[t1m] * WCOLS + (sd // P) * K1 + j[t1m]) * P + (sd % P)
        gl = np.full(RCOLS * P, NTAB - 1, np.int32)
        rr = np.zeros((RCOLS * P, 3), FP)
        rr[:, 0] = 100.0
        gl[pos1] = slot[src[em[t1m]]]
        rr[pos1] = r_in[em[t1m]]
        epos_all[em[t1m]] = pos1
        # overflow: pair-aligned fill per window into the uniform chunks.
        # pair k of a chunk sits at partition k%128, cols (2*(k//128),
        # 2*(k//128)+1); scatter idx is per pair.
        sxp = np.full((OVTOT // 2) * P, NPADU, np.int32)   # trash row pad
        emo, ewo_, edo_ = em[ovm], ew_[ovm], ed_[ovm]
        for w_ in range(NCORES):
            sel = ewo_ == w_
            eids = emo[sel]
            dsts = edo_[sel]
            n = len(eids)
            if n:
                # build per-pair (edge_a, edge_b(-1), dst) lists
                chg2 = np.empty(n, bool)
                chg2[0] = True
                chg2[1:] = dsts[1:] != dsts[:-1]
                g0 = np.where(chg2, np.arange(n), 0)
                np.maximum.accumulate(g0, out=g0)
                r2 = np.arange(n) - g0          # rank within cell
                pe = np.nonzero(r2 % 2 == 0)[0]  # pair-leader edge idx
                pa = eids[pe]
                pb = np.full(len(pe), -1, np.int64)
                hasb = (pe + 1 < n) & (np.append(
                    dsts[1:] == dsts[:-1], False)[pe])
                pb[hasb] = eids[pe[hasb] + 1]
                pdst = dsts[pe]
            else:
                pa = pb = pdst = np.zeros(0, np.int64)
            npair = len(pa)
            off = 0
            for ci in wchunks[w_]:
                cap = ovsched[ci][1] * P // 2
                take = min(npair - off, cap)
                if take <= 0:
                    break
                cbase = T1COLS + ovbase[ci]
                k = np.arange(take)
                pp, tt = k % P, k // P
                pos_a = (cbase + 2 * tt) * P + pp
                pos_b = (cbase + 2 * tt + 1) * P + pp
                ea = pa[off:off + take]
                gl[pos_a] = slot[src[ea]]
                rr[pos_a] = r_in[ea]
                epos_all[ea] = pos_a
                eb = pb[off:off + take]
                mb = eb >= 0
                gl[pos_b[mb]] = slot[src[eb[mb]]]
                rr[pos_b[mb]] = r_in[eb[mb]]
                epos_all[eb[mb]] = pos_b[mb]
                pbase = (ovbase[ci] // 2) * P
                sxp[pbase + k] = slot[pdst[off:off + take]]
                off += take
            assert off == npair, (off, npair)
        cores.append(dict(gl=gl, sxp=sxp, rr=rr))

    # tier1 chunk col spans within each window region
    t1spans = []
    a = 0
    while a < WCOLS:
        t1spans.append((a, min(a + CPT, WCOLS)))
        a = min(a + CPT, WCOLS)

    return dict(NPADU=NPADU, COLS=COLS, NTAB=NTAB, WCOLS=WCOLS,
                T1COLS=T1COLS, RCOLS=RCOLS, OVTOT=OVTOT,
                t1spans=t1spans, ovsched=ovsched, ovbase=ovbase,
                cores=cores, slot=slot, corenode=corenode, first=first,
                outdeg=outdeg, indeg=indeg, epos_all=epos_all,
                src=src, dst=dst, gid=gid, cnt_core=cnt_core)


def pack_core_inputs(plan):
    """Per-core GIX/SIX (wrapped idx) and R1 streams."""
    RCOLS, T1COLS = plan["RCOLS"], plan["T1COLS"]
    packs = []
    for c in range(NCORES):
        cc = plan["cores"][c]
        gixw = wrap_idx16(cc["gl"], RCOLS * P)        # [128, RCOLS*8]
        sixw = wrap_idx16(cc["sxp"], (plan["OVTOT"] // 2) * P)
        r1 = (cc["rr"].reshape(RCOLS, P, 3).transpose(1, 0, 2)
              .reshape(P, RCOLS * 3))
        packs.append(dict(GIX=gixw, SIX=sixw, R1=r1))
    return packs


def nodearr(plan, vals, c):
    COLS = plan["COLS"]
    out = np.zeros((P, COLS), vals.dtype)
    m = plan["corenode"] == c
    s = plan["slot"][m]
    out[s % P, s // P] = vals[m]
    return out
